# revision 1
# baseline (speedup 1.0000x reference)
"""Trainium2 Bass kernel for nn_CausalSelfAttention_27676769255613.

Self-contained: builds an 8-core SPMD Bass/Tile kernel, shards the full
inputs on the host, runs via run_bass_kernel_spmd, and reassembles the
full output.

Sharding: tensor-parallel over heads (2 heads per core); Wq/Wk/Wv
column-split, Wproj row-split with an on-device AllGather of the per-head
attention outputs so each core computes a 256-column slice of the final
projection (+ residual).

Layout strategy (per core, heads h0=2c, h1=2c+1):
  - x passed pre-transposed (xT: C x T); weight slices pre-transposed.
  - q,k produced directly in (D, T) layout (head-dim on partitions):
    exactly the lhsT/rhs layout the score matmul wants.
  - v produced in (T, D) layout: exactly the PV-matmul lhsT layout.
  - scores computed transposed (k on partitions, q free) so the softmax
    bias cs[q]-cs[k] splits into a PE rank-1 broadcast (+cs_q) and an
    ACT per-partition bias (-cs_k); exp needs no running max because
    true (biased, causal) scores are bounded.
  - rms-norm, rope pair-flip and row-broadcasts use ones-column,
    permutation and rank-1 matmuls on the PE instead of partition
    reductions/shuffles.
"""

import math
import threading

import ml_dtypes
import numpy as np

import concourse.bass as bass
import concourse.tile as tile
from concourse import bacc, mybir
from concourse.bass_utils import run_bass_kernel_spmd

F32 = mybir.dt.float32
BF16 = mybir.dt.bfloat16

# Problem dims (hardcoded per spec).
B, T, C, H = 1, 2048, 2048, 16
D = C // H              # 128 head dim
NCORES = 8
HPC = H // NCORES       # 2 heads per core
F = HPC * D             # 256 per-core features
KC = C // 128           # 16 contraction chunks
TCH = T // 128          # 16 token chunks
NQT = T // 512          # 4 q-tiles of 512
NQUARTER = 4            # projection T-quarters (512 cols each)
NEG = -1.0e30
ROPE_BASE = 10000.0


def _build_program():
    nc = bacc.Bacc(num_devices=NCORES)

    xT = nc.dram_tensor("xT", [C, T], BF16, kind="ExternalInput")
    wqT = nc.dram_tensor("wqT", [C, F], BF16, kind="ExternalInput")
    wkT = nc.dram_tensor("wkT", [C, F], BF16, kind="ExternalInput")
    wvld_d = nc.dram_tensor("wvld", [C, F + HPC], BF16, kind="ExternalInput")
    wpT = nc.dram_tensor("wpT", [C, F], BF16, kind="ExternalInput")
    v1r = nc.dram_tensor("v1r", [128, TCH, F], F32, kind="ExternalInput")
    resr = nc.dram_tensor("resr", [128, TCH, F], F32, kind="ExternalInput")
    qcos_d = nc.dram_tensor("qcos", [D, T], BF16, kind="ExternalInput")
    qsin_d = nc.dram_tensor("qsin", [D, T], BF16, kind="ExternalInput")
    kcos_d = nc.dram_tensor("kcos", [D, T], BF16, kind="ExternalInput")
    ksin_d = nc.dram_tensor("ksin", [D, T], BF16, kind="ExternalInput")
    xq_d = nc.dram_tensor("xq", [D, HPC], F32, kind="ExternalInput")
    xk_d = nc.dram_tensor("xk", [D, HPC], F32, kind="ExternalInput")
    xv_d = nc.dram_tensor("xv", [1, F], F32, kind="ExternalInput")
    amask_d = nc.dram_tensor("amask", [128, 4, 512], BF16, kind="ExternalInput")
    perm_d = nc.dram_tensor("perm", [128, 128], F32, kind="ExternalInput")
    esel_d = nc.dram_tensor("esel", [HPC, HPC, 128], F32, kind="ExternalInput")
    out = nc.dram_tensor("out", [T, F], F32, kind="ExternalOutput")

    eps = float(np.finfo(np.float32).eps)
    from contextlib import ExitStack

    with tile.TileContext(nc) as tc:
      with (
          tc.tile_pool(name="dram", bufs=1, space="DRAM") as DRP,
          tc.tile_pool(name="p7w", bufs=1) as P7W,
      ):
        yT_dram = DRP.tile([HPC, 2, D, 1024], BF16, tag="ytd")
        yghalf = [[DRP.tile([NCORES, D, 1024], BF16, tag=f"ygh{h}{tb}",
                            name=f"ygh{h}{tb}", addr_space="Shared")
                   for tb in range(2)] for h in range(HPC)]
        p7t = {}
        with tc.tile_pool(name="always", bufs=1) as AL:
            # ---- persistent SBUF ----
            vbuf = AL.tile([128, TCH, F], BF16, tag="vbuf")
            qTb = [AL.tile([D, T], BF16, tag=f"qTb{h}", name=f"qTb{h}") for h in range(HPC)]
            kTb = [AL.tile([D, T], BF16, tag=f"kTb{h}", name=f"kTb{h}") for h in range(HPC)]
            onescol_b = AL.tile([128, 1], BF16, tag="onescol_b")
            ldbuf = AL.tile([128, TCH, HPC], F32, tag="ldbuf")
            csq = [AL.tile([128, T], F32, tag=f"csq{h}", name=f"csq{h}") for h in range(HPC)]
            negcs = AL.tile([128, TCH, HPC], F32, tag="negcs")
            csrow = AL.tile([HPC, T], F32, tag="csrow")

            onescol = AL.tile([128, 1], F32, tag="onescol")
            nc.vector.memset(onescol, 1.0)
            nc.vector.memset(onescol_b, 1.0)
            onesrow = AL.tile([1, 128], F32, tag="onesrow")
            nc.vector.memset(onesrow, 1.0)
            perm = AL.tile([128, 128], F32, tag="perm")
            nc.sync.dma_start(out=perm, in_=perm_d[:, :])
            amask = AL.tile([128, 4, 512], BF16, tag="amask")
            nc.sync.dma_start(out=amask, in_=amask_d[:, :, :])
            xq_sb = AL.tile([D, HPC], F32, tag="xq")
            xk_sb = AL.tile([D, HPC], F32, tag="xk")
            nc.sync.dma_start(out=xq_sb, in_=xq_d[:, :])
            nc.sync.dma_start(out=xk_sb, in_=xk_d[:, :])
            xv_sb = AL.tile([128, F], F32, tag="xv")
            nc.sync.dma_start(
                out=xv_sb,
                in_=bass.AP(tensor=xv_d, offset=0, ap=[[0, 128]] + xv_d.ap().ap[1:]),
            )

            # ======== Phase 1: fused q/k/v/ld projections (T quarters) ========
            with (
                tc.tile_pool(name="wqkv", bufs=1) as WQ,
                tc.tile_pool(name="xs", bufs=1) as XS,
                tc.tile_pool(name="pjps", bufs=1, space="PSUM") as PJ,
                tc.tile_pool(name="pstr", bufs=3) as PS1,
            ):
                wq_sb = WQ.tile([128, KC, F], BF16, tag="wq")
                wk_sb = WQ.tile([128, KC, F], BF16, tag="wk")
                wv_sb = WQ.tile([128, KC, F + HPC], BF16, tag="wv")
                for lohi in range(2):
                    ks = slice(8 * lohi, 8 * (lohi + 1))
                    nc.sync.dma_start(
                        out=wq_sb[:, ks, :],
                        in_=wqT.ap().rearrange("(k p) m -> p k m", p=128)[:, ks, :])
                    nc.sync.dma_start(
                        out=wk_sb[:, ks, :],
                        in_=wkT.ap().rearrange("(k p) m -> p k m", p=128)[:, ks, :])
                    nc.sync.dma_start(
                        out=wv_sb[:, ks, :],
                        in_=wvld_d.ap().rearrange("(k p) m -> p k m", p=128)[:, ks, :])

                braw = None
                for quarter in range(NQUARTER):
                    t0 = quarter * 512
                    with nc.named_scope(f"proj{quarter}"):
                        xbuf = XS.tile([128, KC, 512], BF16, tag="xbuf", bufs=2)
                        nc.sync.dma_start(
                            out=xbuf,
                            in_=xT.ap().rearrange("(k p) t -> p k t", p=128)[:, :, t0:t0 + 512],
                        )
                        pq = [PJ.tile([128, 512], F32, tag=f"pq{m}", name=f"pq{m}")
                              for m in range(HPC)]
                        pk = [PJ.tile([128, 512], F32, tag=f"pk{m}", name=f"pk{m}")
                              for m in range(HPC)]
                        pv = [PJ.tile([128, F + HPC], F32, tag=f"pv{m}", name=f"pv{m}")
                              for m in range(4)]
                        for kc in range(KC):
                            st, sp = kc == 0, kc == KC - 1
                            rhs = xbuf[:, kc, :]
                            for m in range(HPC):
                                nc.tensor.matmul(
                                    pq[m], wq_sb[:, kc, 128 * m:128 * (m + 1)], rhs,
                                    start=st, stop=sp)
                                nc.tensor.matmul(
                                    pk[m], wk_sb[:, kc, 128 * m:128 * (m + 1)], rhs,
                                    start=st, stop=sp)
                            for lm in range(4):
                                nc.tensor.matmul(
                                    pv[lm], xbuf[:, kc, 128 * lm:128 * (lm + 1)],
                                    wv_sb[:, kc, :], start=st, stop=sp)
                        for m in range(HPC):
                            dst = slice(t0, t0 + 512)
                            nc.scalar.copy(qTb[m][:, dst], pq[m])
                            nc.scalar.copy(kTb[m][:, dst], pk[m])
                        v1q = PS1.tile([128, 4, F], F32, tag="v1q", bufs=2)
                        nc.sync.dma_start(out=v1q, in_=v1r[:, 4 * quarter:4 * quarter + 4, :])
                        for lm in range(4):
                            ch = 4 * quarter + lm
                            nc.vector.tensor_add(vbuf[:, ch, :], pv[lm][:, :F], v1q[:, lm, :])
                            nc.scalar.copy(ldbuf[:, ch, :], pv[lm][:, F:F + HPC])
                        # v token-shift for this quarter's chunks
                        c0 = 4 * quarter
                        vprev = PS1.tile([128, 4, F], BF16, tag="vprev", bufs=2)
                        nc.sync.dma_start(out=vprev[1:128, :, :],
                                          in_=vbuf[0:127, c0:c0 + 4, :])
                        nc.sync.dma_start(out=vprev[0:1, 1:4, :],
                                          in_=vbuf[127:128, c0:c0 + 3, :])
                        if quarter == 0:
                            nc.sync.dma_start(out=vprev[0:1, 0:1, :],
                                              in_=vbuf[0:1, 0:1, :])
                        else:
                            nc.sync.dma_start(out=vprev[0:1, 0:1, :], in_=braw)
                        nbraw = PS1.tile([1, F], BF16, tag="braw", bufs=2)
                        nc.sync.dma_start(out=nbraw, in_=vbuf[127:128, c0 + 3, :])
                        braw = nbraw
                        nc.vector.tensor_sub(vprev, vprev, vbuf[:, c0:c0 + 4, :])
                        xvb = bass.AP(tensor=xv_sb.tensor, offset=xv_sb.offset,
                                      ap=[list(xv_sb.ap[0]), [0, 4], list(xv_sb.ap[1])])
                        nc.vector.tensor_mul(vprev, vprev, xvb)
                        nc.vector.tensor_add(vbuf[:, c0:c0 + 4, :],
                                             vbuf[:, c0:c0 + 4, :], vprev)

            # ======== Phases 4+5 (+3 inline): per-head norm + attention ========
            with (
                tc.tile_pool(name="at5", bufs=3) as A5,
                tc.tile_pool(name="at5y", bufs=2) as A5Y,
                tc.tile_pool(name="at5ps", bufs=1, space="PSUM") as A5P,
            ):
                es = ExitStack()
                RP = es.enter_context(tc.tile_pool(name="rope", bufs=1))
                Q4 = es.enter_context(tc.tile_pool(name="qk4", bufs=1))
                Q4P = es.enter_context(tc.tile_pool(name="qk4ps", bufs=1, space="PSUM"))
                eps_sb = RP.tile([128, 1], F32, tag="eps")
                nc.vector.memset(eps_sb, eps)
                qcos = RP.tile([D, T], BF16, tag="qcos")
                qsin = RP.tile([D, T], BF16, tag="qsin")
                kcos = RP.tile([D, T], BF16, tag="kcos")
                ksin = RP.tile([D, T], BF16, tag="ksin")
                for dst, srct in ((qcos, qcos_d), (qsin, qsin_d),
                                  (kcos, kcos_d), (ksin, ksin_d)):
                    nc.sync.dma_start(out=dst, in_=srct[:, :])

                esel_sb = RP.tile([HPC, HPC, 128], F32, tag="esel")
                nc.sync.dma_start(out=esel_sb, in_=esel_d[:, :, :])
                with tc.tile_pool(name="dk", bufs=1) as DK:
                    with nc.named_scope("decay"):
                        nc.scalar.activation(ldbuf, ldbuf,
                                             mybir.ActivationFunctionType.Sigmoid)
                        nc.scalar.activation(ldbuf, ldbuf,
                                             mybir.ActivationFunctionType.Ln)
                        ldsc = DRP.tile([HPC, T], BF16, tag="ldsc")
                        nsc = DRP.tile([HPC, T], F32, tag="nsc")
                        lsb16 = DK.tile([128, TCH, HPC], BF16, tag="lsb16")
                        nc.vector.tensor_copy(lsb16, ldbuf)
                        for hh in range(HPC):
                            nc.sync.dma_start(
                                out=ldsc[hh].rearrange("(c p) -> p c", p=128),
                                in_=lsb16[:, :, hh],
                            )
                        ldrow = DK.tile([HPC, T], BF16, tag="ldrow")
                        nc.sync.dma_start(out=ldrow, in_=ldsc[:, :])
                        nc.vector.memset(csrow[:, 0:1], 0.0)
                        nc.vector.tensor_tensor_scan(
                            csrow[:, 1:T], ldrow[:, 0:T - 1], ldrow[:, 0:T - 1],
                            initial=0.0,
                            op0=mybir.AluOpType.add, op1=mybir.AluOpType.bypass)
                        for hh in range(HPC):
                            nc.sync.dma_start(out=nsc[hh], in_=csrow[hh:hh + 1, :])
                            nc.sync.dma_start(
                                out=negcs[:, :, hh],
                                in_=nsc[hh].rearrange("(c p) -> p c", p=128),
                            )
                        nc.vector.tensor_scalar_mul(negcs, negcs, -1.0)

                for h in range(HPC):
                    # ---- norm/shift/rope for q_h, k_h ----
                    for tenb, xmix, cosT, sinT in (
                        (qTb, xq_sb, qcos, qsin),
                        (kTb, xk_sb, kcos, ksin),
                    ):
                        nm = f"{'q' if tenb is qTb else 'k'}{h}"
                        with nc.named_scope(f"norm_{nm}"):
                            a = tenb[h]
                            sq = Q4.tile([D, T], F32, tag="sq", bufs=2)
                            nc.scalar.square(sq, a)
                            qn = Q4.tile([D, T], F32, tag="qn", bufs=1)
                            for n in range(NQT):
                                ps = Q4P.tile([1, 512], F32, tag="ps")
                                nc.tensor.matmul(ps, onescol,
                                                 sq[:, 512 * n:512 * (n + 1)],
                                                 start=True, stop=True)
                                rr = Q4.tile([1, 512], F32, tag="rr", bufs=2)
                                nc.scalar.activation(
                                    rr, ps,
                                    mybir.ActivationFunctionType.Abs_reciprocal_sqrt,
                                    bias=eps_sb[0:1, :], scale=1.0 / D)
                                pb2 = Q4P.tile([128, 512], F32, tag="pb2")
                                nc.tensor.matmul(pb2, onesrow, rr,
                                                 start=True, stop=True)
                                nc.vector.tensor_mul(
                                    qn[:, 512 * n:512 * (n + 1)],
                                    a[:, 512 * n:512 * (n + 1)], pb2)
                            dif = Q4.tile([D, T], F32, tag="dif", bufs=1)
                            nc.vector.memset(dif[:, 0:1], 0.0)
                            nc.vector.tensor_sub(dif[:, 1:T], qn[:, 0:T - 1], qn[:, 1:T])
                            qs = sq  # reuse
                            nc.vector.scalar_tensor_tensor(
                                qs, dif, xmix[:, h:h + 1], qn,
                                op0=mybir.AluOpType.mult, op1=mybir.AluOpType.add)
                            m1 = qn  # reuse
                            nc.vector.tensor_mul(m1, qs, cosT)
                            for n in range(NQT):
                                pf = Q4P.tile([128, 512], F32, tag="pf")
                                nc.tensor.matmul(pf, perm, qs[:, 512 * n:512 * (n + 1)],
                                                 start=True, stop=True)
                                nc.vector.tensor_mul(dif[:, 512 * n:512 * (n + 1)], pf,
                                                     sinT[:, 512 * n:512 * (n + 1)])
                            nc.vector.tensor_add(tenb[h], m1, dif)

                    if h == 0:
                        with nc.named_scope("decaymm"):
                            for hh in range(HPC):
                                eh = esel_sb[:, hh, :]
                                for n in range(NQT):
                                    pb = Q4P.tile([128, 512], F32, tag="pb2")
                                    nc.tensor.matmul(pb, eh,
                                                     csrow[:, 512 * n:512 * (n + 1)],
                                                     start=True, stop=True)
                                    nc.scalar.copy(csq[hh][:, 512 * n:512 * (n + 1)], pb)
                    else:
                        # frees rope/norm pools; prefetch out-proj weights
                        es.close()
                        p7t["wp"] = P7W.tile([128, KC, F], BF16, tag="wp", name="wp_sb")
                        nc.gpsimd.dma_start(out=p7t["wp"],
                                            in_=wpT.ap().rearrange("(k p) m -> p k m", p=128))
                        p7t["resl"] = P7W.tile([128, TCH, F], F32, tag="resl", name="resl")
                        nc.gpsimd.dma_start(out=p7t["resl"], in_=resr[:, :, :])

                    # ---- attention for head h ----
                    with nc.named_scope(f"attn{h}"):
                        yTh = A5Y.tile([D, T], BF16, tag="yTh")
                        for n in range(NQT):
                            qsl = slice(512 * n, 512 * (n + 1))
                            yps = A5P.tile([128, 512], F32, tag="yps", bufs=1)
                            zps = A5P.tile([1, 512], F32, tag="zps", bufs=1)
                            nj = 4 * n + 4
                            pend = None
                            for j in range(nj):
                                stp = A5P.tile([128, 512], F32, tag="stp", bufs=2)
                                nc.tensor.matmul(stp, kTb[h][:, 128 * j:128 * (j + 1)],
                                                 qTb[h][:, qsl], start=True, stop=True)
                                if pend is not None:
                                    nc.tensor.matmul(yps,
                                                     vbuf[:, pend[0], 128 * h:128 * (h + 1)],
                                                     pend[1], start=(pend[0] == 0), stop=False)
                                    nc.tensor.matmul(zps, onescol_b, pend[1],
                                                     start=(pend[0] == 0), stop=False)
                                xsb = A5.tile([128, 512], F32, tag="xsb", bufs=2)
                                nc.vector.tensor_add(xsb, stp, csq[h][:, qsl])
                                if j // 4 == n:
                                    nc.vector.tensor_add(xsb, xsb, amask[:, j % 4, :])
                                esb = A5.tile([128, 512], BF16, tag="esb")
                                nc.scalar.activation(esb, xsb,
                                                     mybir.ActivationFunctionType.Exp,
                                                     bias=negcs[:, j, h:h + 1])
                                pend = (j, esb)
                            nc.tensor.matmul(yps, vbuf[:, pend[0], 128 * h:128 * (h + 1)],
                                             pend[1], start=(pend[0] == 0), stop=True)
                            nc.tensor.matmul(zps, onescol_b, pend[1],
                                             start=(pend[0] == 0), stop=True)
                            zl = A5.tile([1, 512], F32, tag="zl")
                            nc.scalar.activation(zl, zps,
                                                 mybir.ActivationFunctionType.Ln)
                            rz = A5.tile([1, 512], F32, tag="rz")
                            nc.scalar.activation(rz, zl,
                                                 mybir.ActivationFunctionType.Exp,
                                                 scale=-1.0)
                            zbp = A5P.tile([128, 512], F32, tag="stp", bufs=2)
                            nc.tensor.matmul(zbp, onesrow, rz, start=True, stop=True)
                            zbs = A5.tile([128, 512], F32, tag="zbs")
                            nc.scalar.copy(zbs, zbp)
                            nc.vector.tensor_mul(yTh[:, qsl], yps, zbs)
                            if n % 2 == 1:
                                tb = n // 2
                                nc.sync.dma_start(
                                    out=yT_dram[h, tb],
                                    in_=yTh[:, 1024 * tb:1024 * (tb + 1)])
                                nc.gpsimd.collective_compute(
                                    "AllGather",
                                    mybir.AluOpType.bypass,
                                    replica_groups=[list(range(NCORES))],
                                    ins=[yT_dram[h, tb]],
                                    outs=[yghalf[h][tb][:, :, :]],
                                )
        # AL closed here
        # ======== Phase 7: output projection + residual ========
        with (
            tc.tile_pool(name="p7", bufs=3) as P7,
            tc.tile_pool(name="p7ps", bufs=1, space="PSUM") as P7P,
        ):
            with nc.named_scope("outproj"):
                wp_sb, resl = p7t["wp"], p7t["resl"]
                for h in range(HPC):
                    for tb in range(2):
                        yg = P7W.tile([128, NCORES, 1024], BF16, tag="yga",
                                      bufs=2, name=f"yg{h}{tb}")
                        nc.sync.dma_start(
                            out=yg,
                            in_=yghalf[h][tb].rearrange("g p t -> p g t"))
                        po = [P7P.tile([128, F], F32, tag=f"po{i}", name=f"po{i}")
                              for i in range(8)]
                        for g in range(NCORES):
                            hh = HPC * g + h
                            for i in range(8):
                                nc.tensor.matmul(
                                    po[i], yg[:, g, 128 * i:128 * (i + 1)],
                                    wp_sb[:, hh, :],
                                    start=(g == 0), stop=(g == NCORES - 1))
                        for i in range(8):
                            m = 8 * tb + i
                            if h == 0:
                                nc.vector.tensor_add(resl[:, m, :], po[i],
                                                     resl[:, m, :])
                            else:
                                ot = P7.tile([128, F], F32, tag="ot")
                                nc.vector.tensor_add(ot, po[i], resl[:, m, :])
                                nc.sync.dma_start(
                                    out=out[128 * m:128 * (m + 1), :], in_=ot)

    nc.compile()
    return nc


_CACHE = {}
_LOCK = threading.Lock()


def _get_program():
    with _LOCK:
        if "nc" not in _CACHE:
            _CACHE["nc"] = _build_program()
        return _CACHE["nc"]


def _rope_tables():
    freq = (1.0 / ROPE_BASE) ** np.linspace(0.0, 1.0, D // 2, dtype=np.float32)
    freq = np.repeat(freq, 2)
    theta = np.arange(T, dtype=np.float32)[:, None] * freq[None, :]
    cos = np.cos(theta).astype(np.float32)
    sin = np.sin(theta).astype(np.float32)
    sin[:, 1::2] *= -1.0
    return np.ascontiguousarray(cos.T), np.ascontiguousarray(sin.T)   # (D, T)


def _host_inputs(residual, x, v1, Wq, Wk, Wv, Wproj, Wd, lamb, x_q, x_k, x_v):
    lam = np.float32(lamb)
    xTf = np.ascontiguousarray(x[0].T.astype(np.float32))       # (C, T)
    cosT, sinT = _rope_tables()
    sc = np.float32(1.0 / math.sqrt(D))
    qcos, qsin = cosT * sc, sinT * sc

    kk = np.arange(128)[:, None]
    qq = np.arange(512)[None, :]
    amask = np.stack(
        [np.where(qq >= 128 * r + kk, 0.0, NEG) for r in range(4)], axis=1
    ).astype(np.float32)                                        # (128, 4, 512)
    permm = np.zeros((128, 128), np.float32)
    permm[np.arange(128), np.arange(128) ^ 1] = 1.0
    esel = np.zeros((HPC, HPC, 128), np.float32)
    for hh in range(HPC):
        esel[hh, hh, :] = 1.0

    in_maps = []
    for c in range(NCORES):
        rs = slice(F * c, F * (c + 1))
        hsel = slice(HPC * c, HPC * (c + 1))
        wvs = ((1.0 - lam) * Wv[rs]).astype(np.float32)          # (F, C)
        wvld = np.concatenate([wvs.T, Wd[hsel].T.astype(np.float32)], axis=1)
        v1s = (lam * v1[0][:, rs]).astype(np.float32)            # (T, F)
        ress = residual[0][:, rs].astype(np.float32)
        in_maps.append({
            "xT": xTf.astype(ml_dtypes.bfloat16),
            "wqT": np.ascontiguousarray(Wq[rs].T).astype(ml_dtypes.bfloat16),
            "wkT": np.ascontiguousarray(Wk[rs].T).astype(ml_dtypes.bfloat16),
            "wvld": np.ascontiguousarray(wvld).astype(ml_dtypes.bfloat16),
            "wpT": np.ascontiguousarray(Wproj[rs].T).astype(ml_dtypes.bfloat16),
            "v1r": np.ascontiguousarray(
                v1s.reshape(TCH, 128, F).transpose(1, 0, 2)),
            "resr": np.ascontiguousarray(
                ress.reshape(TCH, 128, F).transpose(1, 0, 2)),
            "qcos": qcos.astype(ml_dtypes.bfloat16), "qsin": qsin.astype(ml_dtypes.bfloat16),
            "kcos": cosT.astype(ml_dtypes.bfloat16), "ksin": sinT.astype(ml_dtypes.bfloat16),
            "xq": np.ascontiguousarray(x_q[hsel].T.astype(np.float32)),
            "xk": np.ascontiguousarray(x_k[hsel].T.astype(np.float32)),
            "xv": np.ascontiguousarray(
                x_v[hsel].reshape(1, F).astype(np.float32)),
            "amask": amask.astype(ml_dtypes.bfloat16),
            "perm": permm,
            "esel": esel,
        })
    return in_maps


def kernel(residual, x, v1, x0, dx0, Wq, Wk, Wv, Wproj, Wd, lamb, x_q, x_k,
           x_v, token_ids, _results_hook=None):
    in_maps = _host_inputs(np.asarray(residual), np.asarray(x), np.asarray(v1),
                           np.asarray(Wq), np.asarray(Wk), np.asarray(Wv),
                           np.asarray(Wproj), np.asarray(Wd), np.asarray(lamb),
                           np.asarray(x_q), np.asarray(x_k), np.asarray(x_v))
    nc = _get_program()
    res = run_bass_kernel_spmd(nc, in_maps, list(range(NCORES)))
    if _results_hook is not None:
        _results_hook(res)
    outp = np.empty((B, T, C), np.float32)
    for c in range(NCORES):
        outp[0][:, F * c:F * (c + 1)] = np.asarray(
            res.results[c]["out"]).reshape(T, F)
    return outp

